# revision 19
# baseline (speedup 1.0000x reference)
"""MetaUpscale (Meta-SR) Trainium2 kernel — fp8(e3m4) weights + column-tiled PE.

out[b,o,i,j] = sum_{c,ky,kx} xpad[b,c,(i//2)+ky,(j//2)+kx] * w[i*OW+j, (c*3+ky)*3+kx, o]

Shapes: x [4,64,96,96] f32, weight [36864, 576, 3] f32 -> out [4,3,192,192] f32.

Strategy (memory-bound: the 255MB weight tensor dominates):
- Shard over output rows: core r handles out rows [24r, 24r+24) i.e. source
  rows a in [12r, 12r+12).
- The weight stream rides as fp8 e3m4 (scaled x128; the extraction mask
  carries the 2^-7 descale): 9 tap rows per group pair — the two K=64
  leftover taps share one K=128 row [wa8; wb8], annihilated on the wrong
  half by zero-padded x slabs (ZL=[plain;0], ZU=[0;plain]) instead of zero
  weights, so no zero bytes ride the stream (~7.96MB/core).  x streams as
  bf16 plain channels only (~0.7MB); shifted/paired slab variants are all
  built on-chip (DVE copies, gpsimd memsets).
- Weight blocks are grouped per PSUM bank (2 pairs = 6912B per partition
  per DMA).  dma_start calls are ordered so the 8 HWDGE completion lanes
  round-robin without blocking any ring mid-stream, and everything
  first-MM-critical avoids the scalar (ACT) ring, which starts ~3us late.
- PE runs uniformly in 128x32 column-tiled mode: every tap matmul is split
  into 4 concurrent 32-column tiles (8 source patches each, rhs N=96, rhs
  dtype fp8e3 moving at bf16 speed).  Leftover-tap matmuls issue after a
  bank's main taps so their ZL/ZU dependencies stay off the critical path.
- The four tiles of 4 consecutive groups share one PSUM bank [128, 4x96]:
  masked extraction runs once per TWO pairs: DVE tensor_mul [128,384]->bf16
  (mask = 2^-7 on active q slots) + reduce over the 8-patch runs -> [128,48].
  The last two banks extract per half as soon as that half's taps finish.
- Outputs for banks 0-6 ride SWDGE (gpsimd); the last two banks' outputs
  ride the HW rings for a short tail.
- LDWEIGHTS overlap (ldw-opt) is enabled via set_compiler_flags.
"""

import numpy as np
import ml_dtypes

import concourse.bacc as bacc
import concourse.mybir as mybir
import concourse.tile as tile
from concourse.bass_utils import run_bass_kernel_spmd

from concourse.compiler_utils import get_compiler_flags, set_compiler_flags
try:
    _flags = get_compiler_flags()
    _patched = [f.replace("--enable-ldw-opt=false", "--enable-ldw-opt=true")
                for f in _flags]
    if _patched != _flags:
        set_compiler_flags(_patched)
except Exception:
    pass

B, C, KS = 4, 64, 3
H = W = 96
OH = OW = 192
NCORES = 8
AROWS = 12            # source rows per core
HS, WS = AROWS + 2, W + 2
NP = 32               # source patches (columns) per group
NCOL = 384            # 4 cgrp x 12 k x 8 q weight columns per tap
NROWS = 9             # 8 paired taps + 1 shared K=64x2 leftover row
NGRP = AROWS * 3      # 36 groups per core (a_loc x j_grp)
NPAIR = NGRP // 2
NBANK = NGRP // 4     # 9 PSUM banks, 4 groups (2 pairs) each
W_SCALE = 128.0       # weights scaled into e3m4's normal range; mask descales

_DT = mybir.dt
_BF = ml_dtypes.bfloat16
_F8 = ml_dtypes.float8_e3m4


def _build_nc():
    dt_mm = _DT.bfloat16
    dt_w = _DT.float8e3
    nc = bacc.Bacc("TRN2", target_bir_lowering=False, debug=False)
    HEAD = 6
    xs_d = nc.dram_tensor("xs", [64, HS, WS, B], dt_mm, kind="ExternalInput").ap()
    wt_d = nc.dram_tensor("wt", [NBANK, 128, 2, NROWS, NCOL], dt_w,
                          kind="ExternalInput").ap()
    mask_d = nc.dram_tensor("mask", [128, NCOL], dt_mm, kind="ExternalInput").ap()
    out_d = nc.dram_tensor("out", [128, NGRP * 12], _DT.float32, kind="ExternalOutput").ap()

    with tile.TileContext(nc) as tc:
        with (
            tc.tile_pool(name="xs", bufs=1) as xs_pool,
            tc.tile_pool(name="msk", bufs=1) as msk_pool,
            tc.tile_pool(name="res", bufs=4) as res_pool,
            tc.tile_pool(name="wt", bufs=1) as wt_pool,
            tc.tile_pool(name="tmp", bufs=3) as tmp_pool,
            tc.tile_pool(name="ps", bufs=4, space="PSUM") as ps_pool,
            tc.tile_pool(name="pslast", bufs=1, space="PSUM") as psl_pool,
        ):
            xh_t = xs_pool.tile([128, HEAD, WS, B], dt_mm, tag="xh")
            thh_t = xs_pool.tile([128, 4, WS, B], dt_mm, tag="thh")
            xt_t = xs_pool.tile([128, HS - HEAD, WS, B], dt_mm, tag="xt")
            tht_t = xs_pool.tile([128, 8, WS, B], dt_mm, tag="tht")
            zl_t = xs_pool.tile([128, AROWS, WS, B], dt_mm, tag="zl")
            zu_t = xs_pool.tile([128, AROWS, WS, B], dt_mm, tag="zu")
            msk_t = msk_pool.tile([128, NCOL], dt_mm)

            wt_tiles = [wt_pool.tile([128, 2, NROWS, NCOL], dt_w, name=f"wt{bk}")
                        for bk in range(NBANK)]

            # dma_start call order == HWDGE completion-lane round-robin
            # order (8 lanes).  Lanes 0-7 go to the first 8 transfers; each
            # later push waits for its lane predecessor, so predecessors
            # are chosen to complete before the issuing engine reaches the
            # push.  The scalar (ACT) ring starts ~3us late, so the
            # first-MM-critical xh/b0 ride sync.
            nc.sync.dma_start(xh_t[0:64, 0:5], xs_d[:, 0:5])           # L0
            nc.scalar.dma_start(msk_t[:], mask_d)                      # L1
            nc.sync.dma_start(wt_tiles[0][:], wt_d[0])                 # L2
            nc.scalar.dma_start(wt_tiles[1][:], wt_d[1])               # L3
            nc.sync.dma_start(xh_t[0:64, 5:HEAD], xs_d[:, 5:HEAD])     # L4
            nc.sync.dma_start(xt_t[0:64, 0:8], xs_d[:, HEAD:HS])       # L5
            nc.sync.dma_start(wt_tiles[2][:], wt_d[2])                 # L6
            nc.scalar.dma_start(wt_tiles[3][:], wt_d[3])               # L7
            nc.sync.dma_start(wt_tiles[4][:], wt_d[4])                 # L0 <- xh1
            nc.scalar.dma_start(wt_tiles[7][:], wt_d[7])               # L1 <- mask
            nc.sync.dma_start(wt_tiles[8][:, 0:1], wt_d[8, :, 0:1])    # L2 <- b0
            nc.scalar.dma_start(wt_tiles[8][:, 1:2], wt_d[8, :, 1:2])  # L3 <- b1
            # Banks 5-6 ride the SWDGE queue: a third descriptor stream
            # fills the SDMA engines' idle slots during the early window,
            # and the HWDGE rings drop to 11 transfers so no push ever
            # waits on a completion lane mid-stream.
            nc.gpsimd.dma_start(wt_tiles[5][:], wt_d[5])
            nc.gpsimd.dma_start(wt_tiles[6][:], wt_d[6])

            # Zero halves of the leftover-tap slabs: gpsimd memset, no
            # deps, runs during the DMA ramp (v6/v7 failed identically
            # with memset vs x0.0-mul, so memset itself is sound).
            nc.gpsimd.memset(zl_t[64:128, :], 0)
            nc.gpsimd.memset(zu_t[0:64, :], 0)

            # On-chip slab builds (DVE), ordered so everything bank-0
            # needs (rows 0-4 + thh + zl/zu heads) lands first: upper
            # halves of xh/xt are the w+1-shifted copies of the plain
            # channels (shifted col 97 is never read); thh/tht pair rows
            # (h, h+1) for the kx=2 taps; zl/zu rows 0..11 hold plain
            # rows 2..13 on one half.
            nc.vector.tensor_copy(xh_t[64:128, 0:5, 0:97], xh_t[0:64, 0:5, 1:98])
            nc.vector.tensor_copy(thh_t[64:128, :], xh_t[0:64, 1:5])
            nc.vector.tensor_copy(thh_t[0:64, :], xh_t[0:64, 0:4])
            nc.vector.tensor_copy(zl_t[0:64, 0:3], xh_t[0:64, 2:5])
            nc.vector.tensor_copy(zu_t[64:128, 0:3], xh_t[0:64, 2:5])
            nc.vector.tensor_copy(xh_t[64:128, 5:HEAD, 0:97], xh_t[0:64, 5:HEAD, 1:98])
            nc.vector.tensor_copy(zl_t[0:64, 3:4], xh_t[0:64, 5:HEAD])
            nc.vector.tensor_copy(zu_t[64:128, 3:4], xh_t[0:64, 5:HEAD])
            nc.vector.tensor_copy(tht_t[0:64, 0:2], xh_t[0:64, 4:HEAD])
            nc.vector.tensor_copy(tht_t[64:128, 0:1], xh_t[0:64, 5:HEAD])
            nc.vector.tensor_copy(xt_t[64:128, 0:8, 0:97], xt_t[0:64, 0:8, 1:98])
            nc.vector.tensor_copy(zl_t[0:64, 4:12], xt_t[0:64, 0:8])
            nc.vector.tensor_copy(zu_t[64:128, 4:12], xt_t[0:64, 0:8])
            nc.vector.tensor_copy(tht_t[0:64, 2:8], xt_t[0:64, 0:6])
            nc.vector.tensor_copy(tht_t[64:128, 1:8], xt_t[0:64, 0:7])

            def xslab(h):
                return (xh_t, h) if h < HEAD else (xt_t, h - HEAD)

            def thslab(a_loc):
                return (thh_t, a_loc) if a_loc < 4 else (tht_t, a_loc - 4)

            def main_taps(ps_t, g, col0, wt_t):
                a_loc, jg = g // 3, g % 3
                halfp = g % 2
                pr = (g // 2) % 2
                for cg in range(4):
                    base = jg * NP + 8 * cg
                    out_ap = ps_t[32 * cg : 32 * cg + 32, col0 : col0 + 96]
                    # 3x K=128: kx=0 on partitions 0-63 (plain slab),
                    # kx=1 on 64-127 (w+1-shifted copy)
                    for ky in range(3):
                        xt_, h = xslab(a_loc + ky)
                        nc.tensor.matmul(
                            out_ap, xt_[:, h, base : base + 8, :],
                            wt_t[:, pr, 5 * halfp + ky, 96 * cg : 96 * cg + 96],
                            start=(ky == 0), stop=False,
                            tile_position=(0, 32 * cg),
                            skip_group_check=True,
                        )
                    # (ky=0,kx=2)+(ky=1,kx=2) via the h-shifted T_H slab
                    th_, ha = thslab(a_loc)
                    nc.tensor.matmul(
                        out_ap, th_[:, ha, base + 2 : base + 10, :],
                        wt_t[:, pr, 5 * halfp + 3, 96 * cg : 96 * cg + 96],
                        start=False, stop=False,
                        tile_position=(0, 32 * cg),
                        skip_group_check=True,
                    )

            def leftover_tap(ps_t, g, col0, wt_t):
                # (ky=2,kx=2): both groups of a pair share weight row 4 =
                # [wa8; wb8]; the foreign half is annihilated by the zero
                # half of ZL (even groups) / ZU (odd groups).
                a_loc, jg = g // 3, g % 3
                pr = (g // 2) % 2
                z_t = zl_t if g % 2 == 0 else zu_t
                for cg in range(4):
                    base = jg * NP + 8 * cg
                    out_ap = ps_t[32 * cg : 32 * cg + 32, col0 : col0 + 96]
                    nc.tensor.matmul(
                        out_ap, z_t[:, a_loc, base + 2 : base + 10, :],
                        wt_t[:, pr, 4, 96 * cg : 96 * cg + 96],
                        start=False, stop=True,
                        tile_position=(0, 32 * cg),
                        skip_group_check=True,
                    )

            def extract(ps_h, col0, ncol, res_cols, out_col, eng):
                suf = "l" if ncol != NCOL else ""
                tmp_t = tmp_pool.tile([128, ncol], dt_mm, name=f"tmp{suf}_t")
                nc.vector.tensor_mul(tmp_t[:], ps_h[:, col0 : col0 + ncol],
                                     msk_t[:, 0:ncol])
                res_t = res_pool.tile([128, res_cols], _DT.float32,
                                      name=f"res{suf}_t")
                nc.vector.reduce_sum(
                    res_t[:],
                    tmp_t[:].rearrange("p (hk q) -> p hk q", q=8),
                    axis=mybir.AxisListType.X,
                )
                eng.dma_start(out_d[:, out_col : out_col + res_cols], res_t[:])

            for bank in range(NBANK):
                last = bank >= NBANK - 2
                wt_t = wt_tiles[bank]
                # NOTE: start=True clears has_written for the whole PSUM
                # bank, so each group's leftover tap must issue before any
                # later group's start=True — keep all 5 taps per group
                # together.
                if last:
                    # Per-group extraction: each group's [128,96] region is
                    # masked/reduced/stored as soon as its taps finish, so
                    # the kernel tail is only the final group's pipeline.
                    ps_a = psl_pool.tile([128, 512], _DT.float32, name=f"ps_a{bank}")
                    ps_b = psl_pool.tile([128, 512], _DT.float32, name=f"ps_b{bank}")
                    for h4 in range(4):
                        ps_h = ps_a if h4 < 2 else ps_b
                        col0 = (h4 % 2) * 96
                        main_taps(ps_h, 4 * bank + h4, col0, wt_t)
                        leftover_tap(ps_h, 4 * bank + h4, col0, wt_t)
                        eng = nc.sync if h4 % 2 == 0 else nc.scalar
                        extract(ps_h, col0, 96, 12, bank * 48 + h4 * 12, eng)
                else:
                    ps_t = ps_pool.tile([128, 512], _DT.float32, name="ps_t")
                    for h4 in range(4):
                        main_taps(ps_t, 4 * bank + h4, h4 * 96, wt_t)
                        leftover_tap(ps_t, 4 * bank + h4, h4 * 96, wt_t)
                    extract(ps_t, 0, NCOL, 48, bank * 48, nc.gpsimd)
    nc.finalize()
    return nc


def _host_prep(x, weight):
    """Returns per-core in_maps for the 8 cores."""
    xpad = np.pad(x, ((0, 0), (0, 0), (1, 1), (1, 1)))
    # [c, h, w, b] so lhsT window columns are contiguous
    xt = np.ascontiguousarray(xpad.transpose(1, 2, 3, 0).astype(_BF))

    # weight [OH*OW, 576, 3] -> [a, di, jg, cgrp, q, dj, c, ky, kx, o]
    w10 = weight.reshape(H, 2, 3, 4, 8, 2, C, KS, KS, 3)
    # -> [a, jg, ky, kx, c, cgrp, di, dj, o, q]   (n = cgrp*96 + k*8 + q)
    wt = np.ascontiguousarray(
        w10.transpose(0, 2, 7, 8, 6, 3, 1, 5, 9, 4).astype(np.float32))
    wt = wt.reshape(H, 3, 9, C, NCOL)

    mask = np.zeros((128, NCOL), dtype=np.float32)
    for m in range(128):
        mask[m, (m // B) % 8 :: 8] = 1.0 / W_SCALE
    mask = mask.astype(_BF)

    in_maps = []
    for r in range(NCORES):
        sl = slice(12 * r, 12 * r + HS)
        xs2 = np.ascontiguousarray(xt[:, sl])
        wtr = wt[AROWS * r : AROWS * (r + 1)].reshape(NGRP, 9, C, NCOL)
        wa = wtr[0::2].reshape(NPAIR, 3, 3, C, NCOL)    # pair ky kx c n
        wb = wtr[1::2].reshape(NPAIR, 3, 3, C, NCOL)
        wtp = np.zeros((NPAIR, 128, NROWS, NCOL), np.float32)
        wtp[:, 0:64, 0:3] = wa[:, :, 0].transpose(0, 2, 1, 3)
        wtp[:, 64:128, 0:3] = wa[:, :, 1].transpose(0, 2, 1, 3)
        wtp[:, 0:64, 3] = wa[:, 0, 2]       # T_H tap for even group
        wtp[:, 64:128, 3] = wa[:, 1, 2]
        wtp[:, 0:64, 4] = wa[:, 2, 2]       # shared row 4 = [wa8; wb8]
        wtp[:, 64:128, 4] = wb[:, 2, 2]
        wtp[:, 0:64, 5:8] = wb[:, :, 0].transpose(0, 2, 1, 3)
        wtp[:, 64:128, 5:8] = wb[:, :, 1].transpose(0, 2, 1, 3)
        wtp[:, 0:64, 8] = wb[:, 0, 2]       # T_H tap for odd group
        wtp[:, 64:128, 8] = wb[:, 1, 2]
        wtp = (wtp * W_SCALE).astype(_F8)
        # [NPAIR, 128, NROWS, NCOL] -> [NBANK, 128, 2, NROWS, NCOL]
        wtp = np.ascontiguousarray(
            wtp.reshape(NBANK, 2, 128, NROWS, NCOL).transpose(0, 2, 1, 3, 4))
        in_maps.append({"xs": xs2, "wt": wtp, "mask": mask})
    return in_maps


def _host_gather(results):
    """results: list of 8 dicts with 'out' [128, 432] -> full [B,3,OH,OW]."""
    res = np.stack([r["out"] for r in results])            # [r, 128, 432]
    res = res.reshape(NCORES, NP, B, AROWS, 3, 2, 2, 3)    # r p b a_loc jg di dj o
    out = res.transpose(2, 7, 0, 3, 5, 4, 1, 6)            # b o r a_loc di jg p dj
    return np.ascontiguousarray(out.reshape(B, 3, OH, OW))


_CACHED_NC = None


def _get_nc():
    global _CACHED_NC
    if _CACHED_NC is None:
        _CACHED_NC = _build_nc()
    return _CACHED_NC


def kernel(x, weight, **run_kwargs):
    x = np.asarray(x, dtype=np.float32)
    weight = np.asarray(weight, dtype=np.float32)
    in_maps = _host_prep(x, weight)
    nc = _get_nc()
    r = run_bass_kernel_spmd(nc, in_maps, core_ids=list(range(NCORES)), **run_kwargs)
    out = _host_gather(r.results)
    kernel.last_result = r
    return out


# revision 21
# speedup vs baseline: 1.1775x; 1.1775x over previous
"""MetaUpscale (Meta-SR) Trainium2 kernel — fp8(e3m4) weights + column-tiled PE.

out[b,o,i,j] = sum_{c,ky,kx} xpad[b,c,(i//2)+ky,(j//2)+kx] * w[i*OW+j, (c*3+ky)*3+kx, o]

Shapes: x [4,64,96,96] f32, weight [36864, 576, 3] f32 -> out [4,3,192,192] f32.

Strategy (memory-bound: the 255MB weight tensor dominates):
- Shard over output rows: core r handles out rows [24r, 24r+24) i.e. source
  rows a in [12r, 12r+12).
- The weight stream rides as fp8 e3m4 (scaled x128; the extraction mask
  carries the 2^-7 descale): 9 tap rows per group pair — the two K=64
  leftover taps share one K=128 row [wa8; wb8], annihilated on the wrong
  half by zero-padded x slabs (ZL=[plain;0], ZU=[0;plain]) instead of zero
  weights, so no zero bytes ride the stream (~7.96MB/core).  x streams as
  bf16 plain channels only (~0.7MB); shifted/paired slab variants are all
  built on-chip (DVE copies, gpsimd memsets).
- Weight blocks are grouped per PSUM bank (2 pairs = 6912B per partition
  per DMA).  dma_start calls are ordered so the 8 HWDGE completion lanes
  round-robin without blocking any ring mid-stream, and everything
  first-MM-critical avoids the scalar (ACT) ring, which starts ~3us late.
- PE runs uniformly in 128x32 column-tiled mode: every tap matmul is split
  into 4 concurrent 32-column tiles (8 source patches each, rhs N=96, rhs
  dtype fp8e3 moving at bf16 speed).  Leftover-tap matmuls issue after a
  bank's main taps so their ZL/ZU dependencies stay off the critical path.
- The four tiles of 4 consecutive groups share one PSUM bank [128, 4x96]:
  masked extraction runs once per TWO pairs: DVE tensor_mul [128,384]->bf16
  (mask = 2^-7 on active q slots) + reduce over the 8-patch runs -> [128,48].
  The last two banks extract per half as soon as that half's taps finish.
- Outputs for banks 0-6 ride SWDGE (gpsimd); the last two banks' outputs
  ride the HW rings for a short tail.
- LDWEIGHTS overlap (ldw-opt) is enabled via set_compiler_flags.
"""

import numpy as np
import ml_dtypes

import concourse.bacc as bacc
import concourse.mybir as mybir
import concourse.tile as tile
from concourse.bass_utils import run_bass_kernel_spmd

from concourse.compiler_utils import get_compiler_flags, set_compiler_flags
try:
    _flags = get_compiler_flags()
    _patched = [f.replace("--enable-ldw-opt=false", "--enable-ldw-opt=true")
                for f in _flags]
    if _patched != _flags:
        set_compiler_flags(_patched)
except Exception:
    pass

B, C, KS = 4, 64, 3
H = W = 96
OH = OW = 192
NCORES = 8
AROWS = 12            # source rows per core
HS, WS = AROWS + 2, W + 2
NP = 32               # source patches (columns) per group
NCOL = 384            # 4 cgrp x 12 k x 8 q weight columns per tap
NROWS = 9             # 8 paired taps + 1 shared K=64x2 leftover row
NGRP = AROWS * 3      # 36 groups per core (a_loc x j_grp)
NPAIR = NGRP // 2
NBANK = NGRP // 4     # 9 PSUM banks, 4 groups (2 pairs) each
W_SCALE = 128.0       # weights scaled into e3m4's normal range; mask descales

_DT = mybir.dt
_BF = ml_dtypes.bfloat16
_F8 = ml_dtypes.float8_e3m4


def _build_nc():
    dt_mm = _DT.bfloat16
    dt_w = _DT.float8e3
    nc = bacc.Bacc("TRN2", target_bir_lowering=False, debug=False)
    HEAD = 6
    xs_d = nc.dram_tensor("xs", [64, HS, WS, B], dt_mm, kind="ExternalInput").ap()
    wt_d = nc.dram_tensor("wt", [NBANK, 128, 2, NROWS, NCOL], dt_w,
                          kind="ExternalInput").ap()
    mask_d = nc.dram_tensor("mask", [128, NCOL], dt_mm, kind="ExternalInput").ap()
    out_d = nc.dram_tensor("out", [128, NGRP * 12], _DT.float32, kind="ExternalOutput").ap()

    with tile.TileContext(nc) as tc:
        with (
            tc.tile_pool(name="xs", bufs=1) as xs_pool,
            tc.tile_pool(name="msk", bufs=1) as msk_pool,
            tc.tile_pool(name="res", bufs=4) as res_pool,
            tc.tile_pool(name="wt", bufs=1) as wt_pool,
            tc.tile_pool(name="tmp", bufs=3) as tmp_pool,
            tc.tile_pool(name="ps", bufs=4, space="PSUM") as ps_pool,
            tc.tile_pool(name="pslast", bufs=1, space="PSUM") as psl_pool,
        ):
            xh_t = xs_pool.tile([128, HEAD, WS, B], dt_mm, tag="xh")
            thh_t = xs_pool.tile([128, 4, WS, B], dt_mm, tag="thh")
            xt_t = xs_pool.tile([128, HS - HEAD, WS, B], dt_mm, tag="xt")
            tht_t = xs_pool.tile([128, 8, WS, B], dt_mm, tag="tht")
            zl_t = xs_pool.tile([128, AROWS, WS, B], dt_mm, tag="zl")
            zu_t = xs_pool.tile([128, AROWS, WS, B], dt_mm, tag="zu")
            msk_t = msk_pool.tile([128, NCOL], dt_mm)

            wt_tiles = [wt_pool.tile([128, 2, NROWS, NCOL], dt_w, name=f"wt{bk}")
                        for bk in range(NBANK)]

            # dma_start call order == HWDGE completion-lane round-robin
            # order (8 lanes).  Lanes 0-7 go to the first 8 transfers; each
            # later push waits for its lane predecessor, so predecessors
            # are chosen to complete before the issuing engine reaches the
            # push.  The scalar (ACT) ring starts ~3us late, so the
            # first-MM-critical xh/b0 ride sync.
            nc.sync.dma_start(xh_t[0:64, 0:5], xs_d[:, 0:5])           # L0
            nc.scalar.dma_start(msk_t[:], mask_d)                      # L1
            nc.sync.dma_start(wt_tiles[0][:, 0:1], wt_d[0, :, 0:1])    # L2
            nc.scalar.dma_start(wt_tiles[0][:, 1:2], wt_d[0, :, 1:2])  # L3
            nc.sync.dma_start(xh_t[0:64, 5:HEAD], xs_d[:, 5:HEAD])     # L4
            nc.sync.dma_start(xt_t[0:64, 0:8], xs_d[:, HEAD:HS])       # L5
            nc.scalar.dma_start(wt_tiles[1][:], wt_d[1])               # L6
            nc.sync.dma_start(wt_tiles[2][:], wt_d[2])                 # L7
            nc.scalar.dma_start(wt_tiles[3][:], wt_d[3])               # L0 <- xh1
            nc.sync.dma_start(wt_tiles[4][:], wt_d[4])                 # L1 <- mask
            nc.scalar.dma_start(wt_tiles[5][:], wt_d[5])               # L2 <- b0p0
            nc.sync.dma_start(wt_tiles[6][:], wt_d[6])                 # L3 <- b0p1
            nc.scalar.dma_start(wt_tiles[7][:], wt_d[7])               # L4 <- xh2
            nc.sync.dma_start(wt_tiles[8][:, 0:1], wt_d[8, :, 0:1])    # L5 <- xt
            nc.scalar.dma_start(wt_tiles[8][:, 1:2], wt_d[8, :, 1:2])  # L6 <- b1

            # Zero halves of the leftover-tap slabs: gpsimd memset, no
            # deps, runs during the DMA ramp (v6/v7 failed identically
            # with memset vs x0.0-mul, so memset itself is sound).
            nc.gpsimd.memset(zl_t[64:128, :], 0)
            nc.gpsimd.memset(zu_t[0:64, :], 0)

            # On-chip slab builds (DVE), ordered so everything bank-0
            # needs (rows 0-4 + thh + zl/zu heads) lands first: upper
            # halves of xh/xt are the w+1-shifted copies of the plain
            # channels (shifted col 97 is never read); thh/tht pair rows
            # (h, h+1) for the kx=2 taps; zl/zu rows 0..11 hold plain
            # rows 2..13 on one half.
            nc.vector.tensor_copy(xh_t[64:128, 0:5, 0:97], xh_t[0:64, 0:5, 1:98])
            nc.vector.tensor_copy(thh_t[64:128, :], xh_t[0:64, 1:5])
            nc.vector.tensor_copy(thh_t[0:64, :], xh_t[0:64, 0:4])
            nc.vector.tensor_copy(zl_t[0:64, 0:3], xh_t[0:64, 2:5])
            nc.vector.tensor_copy(zu_t[64:128, 0:3], xh_t[0:64, 2:5])
            nc.vector.tensor_copy(xh_t[64:128, 5:HEAD, 0:97], xh_t[0:64, 5:HEAD, 1:98])
            nc.vector.tensor_copy(zl_t[0:64, 3:4], xh_t[0:64, 5:HEAD])
            nc.vector.tensor_copy(zu_t[64:128, 3:4], xh_t[0:64, 5:HEAD])
            nc.vector.tensor_copy(tht_t[0:64, 0:2], xh_t[0:64, 4:HEAD])
            nc.vector.tensor_copy(tht_t[64:128, 0:1], xh_t[0:64, 5:HEAD])
            nc.vector.tensor_copy(xt_t[64:128, 0:8, 0:97], xt_t[0:64, 0:8, 1:98])
            nc.vector.tensor_copy(zl_t[0:64, 4:12], xt_t[0:64, 0:8])
            nc.vector.tensor_copy(zu_t[64:128, 4:12], xt_t[0:64, 0:8])
            nc.vector.tensor_copy(tht_t[0:64, 2:8], xt_t[0:64, 0:6])
            nc.vector.tensor_copy(tht_t[64:128, 1:8], xt_t[0:64, 0:7])

            def xslab(h):
                return (xh_t, h) if h < HEAD else (xt_t, h - HEAD)

            def thslab(a_loc):
                return (thh_t, a_loc) if a_loc < 4 else (tht_t, a_loc - 4)

            def main_taps(ps_t, g, col0, wt_t):
                a_loc, jg = g // 3, g % 3
                halfp = g % 2
                pr = (g // 2) % 2
                for cg in range(4):
                    base = jg * NP + 8 * cg
                    out_ap = ps_t[32 * cg : 32 * cg + 32, col0 : col0 + 96]
                    # 3x K=128: kx=0 on partitions 0-63 (plain slab),
                    # kx=1 on 64-127 (w+1-shifted copy)
                    for ky in range(3):
                        xt_, h = xslab(a_loc + ky)
                        nc.tensor.matmul(
                            out_ap, xt_[:, h, base : base + 8, :],
                            wt_t[:, pr, 5 * halfp + ky, 96 * cg : 96 * cg + 96],
                            start=(ky == 0), stop=False,
                            tile_position=(0, 32 * cg),
                            skip_group_check=True,
                        )
                    # (ky=0,kx=2)+(ky=1,kx=2) via the h-shifted T_H slab
                    th_, ha = thslab(a_loc)
                    nc.tensor.matmul(
                        out_ap, th_[:, ha, base + 2 : base + 10, :],
                        wt_t[:, pr, 5 * halfp + 3, 96 * cg : 96 * cg + 96],
                        start=False, stop=False,
                        tile_position=(0, 32 * cg),
                        skip_group_check=True,
                    )

            def leftover_tap(ps_t, g, col0, wt_t):
                # (ky=2,kx=2): both groups of a pair share weight row 4 =
                # [wa8; wb8]; the foreign half is annihilated by the zero
                # half of ZL (even groups) / ZU (odd groups).
                a_loc, jg = g // 3, g % 3
                pr = (g // 2) % 2
                z_t = zl_t if g % 2 == 0 else zu_t
                for cg in range(4):
                    base = jg * NP + 8 * cg
                    out_ap = ps_t[32 * cg : 32 * cg + 32, col0 : col0 + 96]
                    nc.tensor.matmul(
                        out_ap, z_t[:, a_loc, base + 2 : base + 10, :],
                        wt_t[:, pr, 4, 96 * cg : 96 * cg + 96],
                        start=False, stop=True,
                        tile_position=(0, 32 * cg),
                        skip_group_check=True,
                    )

            def extract(ps_h, col0, ncol, res_cols, out_col, eng):
                suf = "l" if ncol != NCOL else ""
                tmp_t = tmp_pool.tile([128, ncol], dt_mm, name=f"tmp{suf}_t")
                nc.vector.tensor_mul(tmp_t[:], ps_h[:, col0 : col0 + ncol],
                                     msk_t[:, 0:ncol])
                res_t = res_pool.tile([128, res_cols], _DT.float32,
                                      name=f"res{suf}_t")
                nc.vector.reduce_sum(
                    res_t[:],
                    tmp_t[:].rearrange("p (hk q) -> p hk q", q=8),
                    axis=mybir.AxisListType.X,
                )
                eng.dma_start(out_d[:, out_col : out_col + res_cols], res_t[:])

            for bank in range(NBANK):
                last = bank >= NBANK - 2
                wt_t = wt_tiles[bank]
                # NOTE: start=True clears has_written for the whole PSUM
                # bank, so each group's leftover tap must issue before any
                # later group's start=True — keep all 5 taps per group
                # together.
                if last:
                    # Per-group extraction: each group's [128,96] region is
                    # masked/reduced/stored as soon as its taps finish, so
                    # the kernel tail is only the final group's pipeline.
                    ps_a = psl_pool.tile([128, 512], _DT.float32, name=f"ps_a{bank}")
                    ps_b = psl_pool.tile([128, 512], _DT.float32, name=f"ps_b{bank}")
                    for h4 in range(4):
                        ps_h = ps_a if h4 < 2 else ps_b
                        col0 = (h4 % 2) * 96
                        main_taps(ps_h, 4 * bank + h4, col0, wt_t)
                        leftover_tap(ps_h, 4 * bank + h4, col0, wt_t)
                        eng = nc.sync if h4 % 2 == 0 else nc.scalar
                        extract(ps_h, col0, 96, 12, bank * 48 + h4 * 12, eng)
                else:
                    ps_t = ps_pool.tile([128, 512], _DT.float32, name="ps_t")
                    for h4 in range(4):
                        main_taps(ps_t, 4 * bank + h4, h4 * 96, wt_t)
                        leftover_tap(ps_t, 4 * bank + h4, h4 * 96, wt_t)
                    extract(ps_t, 0, NCOL, 48, bank * 48, nc.gpsimd)
    nc.finalize()
    return nc


def _host_prep(x, weight):
    """Returns per-core in_maps for the 8 cores."""
    xpad = np.pad(x, ((0, 0), (0, 0), (1, 1), (1, 1)))
    # [c, h, w, b] so lhsT window columns are contiguous
    xt = np.ascontiguousarray(xpad.transpose(1, 2, 3, 0).astype(_BF))

    # weight [OH*OW, 576, 3] -> [a, di, jg, cgrp, q, dj, c, ky, kx, o]
    w10 = weight.reshape(H, 2, 3, 4, 8, 2, C, KS, KS, 3)
    # -> [a, jg, ky, kx, c, cgrp, di, dj, o, q]   (n = cgrp*96 + k*8 + q)
    wt = np.ascontiguousarray(
        w10.transpose(0, 2, 7, 8, 6, 3, 1, 5, 9, 4).astype(np.float32))
    wt = wt.reshape(H, 3, 9, C, NCOL)

    mask = np.zeros((128, NCOL), dtype=np.float32)
    for m in range(128):
        mask[m, (m // B) % 8 :: 8] = 1.0 / W_SCALE
    mask = mask.astype(_BF)

    in_maps = []
    for r in range(NCORES):
        sl = slice(12 * r, 12 * r + HS)
        xs2 = np.ascontiguousarray(xt[:, sl])
        wtr = wt[AROWS * r : AROWS * (r + 1)].reshape(NGRP, 9, C, NCOL)
        wa = wtr[0::2].reshape(NPAIR, 3, 3, C, NCOL)    # pair ky kx c n
        wb = wtr[1::2].reshape(NPAIR, 3, 3, C, NCOL)
        wtp = np.zeros((NPAIR, 128, NROWS, NCOL), np.float32)
        wtp[:, 0:64, 0:3] = wa[:, :, 0].transpose(0, 2, 1, 3)
        wtp[:, 64:128, 0:3] = wa[:, :, 1].transpose(0, 2, 1, 3)
        wtp[:, 0:64, 3] = wa[:, 0, 2]       # T_H tap for even group
        wtp[:, 64:128, 3] = wa[:, 1, 2]
        wtp[:, 0:64, 4] = wa[:, 2, 2]       # shared row 4 = [wa8; wb8]
        wtp[:, 64:128, 4] = wb[:, 2, 2]
        wtp[:, 0:64, 5:8] = wb[:, :, 0].transpose(0, 2, 1, 3)
        wtp[:, 64:128, 5:8] = wb[:, :, 1].transpose(0, 2, 1, 3)
        wtp[:, 0:64, 8] = wb[:, 0, 2]       # T_H tap for odd group
        wtp[:, 64:128, 8] = wb[:, 1, 2]
        wtp = (wtp * W_SCALE).astype(_F8)
        # [NPAIR, 128, NROWS, NCOL] -> [NBANK, 128, 2, NROWS, NCOL]
        wtp = np.ascontiguousarray(
            wtp.reshape(NBANK, 2, 128, NROWS, NCOL).transpose(0, 2, 1, 3, 4))
        in_maps.append({"xs": xs2, "wt": wtp, "mask": mask})
    return in_maps


def _host_gather(results):
    """results: list of 8 dicts with 'out' [128, 432] -> full [B,3,OH,OW]."""
    res = np.stack([r["out"] for r in results])            # [r, 128, 432]
    res = res.reshape(NCORES, NP, B, AROWS, 3, 2, 2, 3)    # r p b a_loc jg di dj o
    out = res.transpose(2, 7, 0, 3, 5, 4, 1, 6)            # b o r a_loc di jg p dj
    return np.ascontiguousarray(out.reshape(B, 3, OH, OW))


_CACHED_NC = None


def _get_nc():
    global _CACHED_NC
    if _CACHED_NC is None:
        _CACHED_NC = _build_nc()
    return _CACHED_NC


def kernel(x, weight, **run_kwargs):
    x = np.asarray(x, dtype=np.float32)
    weight = np.asarray(weight, dtype=np.float32)
    in_maps = _host_prep(x, weight)
    nc = _get_nc()
    r = run_bass_kernel_spmd(nc, in_maps, core_ids=list(range(NCORES)), **run_kwargs)
    out = _host_gather(r.results)
    kernel.last_result = r
    return out
